# revision 5
# baseline (speedup 1.0000x reference)
"""Coord2HeatmapNet Trainium2 kernel.

out[b,c,j,i] = 10*exp(-(((i+.5)/128 - x)^2 + ((j+.5)/128 - y)^2) / (2*(2/128)^2))

Exploited structure:
  * Separable: each heatmap = fy[j] (x) fx[i] outer product.
  * fp32 exp underflows to exactly 0 beyond ~29 px from the peak -> only a
    64-row window per heatmap is nonzero; the pre-zeroed output buffer keeps
    the rest at 0.
  * Derivative_Erf activation = 2/sqrt(pi)*exp(-t^2): one ScalarE op per
    gaussian factor vector.
  * Layout: one heatmap per PARTITION. Partition p of group g holds the whole
    64x128 window of heatmap k=g*128+p as 8192 contiguous floats. The outer
    product is one DVE tensor_tensor with stride-0 broadcasts; the write-out
    is ONE indirect scatter DMA per group (one offset per partition, 32KB
    contiguous per heatmap at its data-dependent window position).
  * coords flat index of x_k is 2k (affine), so per-partition coords load is
    a plain strided DMA.

Sharding: pure data parallel, 8 batches per core across 8 NeuronCores.
"""
import sys

for _p in ("/opt/trn_rl_repo", "/root/.axon_site", "/root/.axon_site/_ro/trn_rl_repo",
           "/root/.axon_site/_ro/pypackages"):
    if _p not in sys.path:
        sys.path.append(_p)

import numpy as np

S = 128
NUM_CLASS = 68
B_TOTAL = 64
N_CORES = 8
B_LOC = B_TOTAL // N_CORES            # 8 batches per core
NHM = B_LOC * NUM_CLASS               # 544 heatmaps per core
WIN = 64                              # window rows per heatmap
NG_FULL = NHM // 128                  # 4 full groups of 128 heatmaps
NG_REM = NHM - NG_FULL * 128          # 32 in the last group
GROUPS = [128] * NG_FULL + ([NG_REM] if NG_REM else [])
FREE = WIN * S                        # 8192 elems (32KB) per heatmap window
SIGMA = 2.0 / S
DENOM = 2.0 * SIGMA * SIGMA           # 1/2048
SINV = float(np.sqrt(1.0 / DENOM))    # 45.254834
A = SINV / S
AMP = float(10.0 * np.pi / 4.0)
OUT_ELEMS = NHM * S * S
RCH = 2                               # DVE product ops per group (r-chunks)

_cache = {}


def _build():
    import concourse.bass as bass
    import concourse.tile as tile
    from concourse import bacc, mybir
    from concourse.bass import IndirectOffsetOnAxis
    from concourse.bass_types import AP

    f32 = mybir.dt.float32
    nc = bacc.Bacc("TRN2", target_bir_lowering=False, debug=False,
                   num_devices=N_CORES)

    coords = nc.dram_tensor("coords", [B_LOC, 2 * NUM_CLASS], f32,
                            kind="ExternalInput")
    out = nc.dram_tensor("out", [OUT_ELEMS], f32, kind="ExternalOutput")
    o2d = out.ap().rearrange("(a b) -> a b", b=1)
    cflat = coords.ap().rearrange("b f -> (b f)")

    derf = mybir.ActivationFunctionType.Derivative_Erf
    op = mybir.AluOpType
    NG = len(GROUPS)

    with tile.TileContext(nc) as tc:
        with tc.tile_pool(name="tabs", bufs=1) as tp, \
             tc.tile_pool(name="main", bufs=6) as mp, \
             tc.tile_pool(name="vecs", bufs=2) as vp:
            # ---- per-heatmap coord tables, partition p = heatmap g*128+p ----
            X2 = tp.tile([128, NG], f32)
            Y2 = tp.tile([128, NG], f32)
            for (t, off) in ((X2, 0), (Y2, 1)):
                # full groups: coords_flat[2*(g*128+p) + off]
                src = AP(tensor=cflat.tensor, offset=off,
                         ap=[[2, 128], [256, NG_FULL]])
                nc.sync.dma_start(t[:, 0:NG_FULL], src)
                if NG_REM:
                    srcr = AP(tensor=cflat.tensor,
                              offset=off + 2 * 128 * NG_FULL,
                              ap=[[2, NG_REM], [256, 1]])
                    nc.sync.dma_start(t[0:NG_REM, NG_FULL:NG], srcr)

            # bias for fx: a/2 - s*x
            BX2 = tp.tile([128, NG], f32)
            nc.vector.tensor_scalar(BX2[:], X2[:], -SINV, A * 0.5,
                                    op.mult, op.add)
            # jo = clamp(rint(128*y) - 32, 0, 64)
            JO2 = tp.tile([128, NG], f32)
            nc.vector.tensor_scalar_mul(JO2[:], Y2[:], float(S))
            JO2I = tp.tile([128, NG], mybir.dt.int32)
            nc.vector.tensor_copy(JO2I[:], JO2[:])
            nc.vector.tensor_copy(JO2[:], JO2I[:])
            nc.vector.tensor_scalar_sub(JO2[:], JO2[:], 32.0)
            nc.vector.tensor_scalar(JO2[:], JO2[:], 0.0, 64.0, op.max, op.min)
            # bias for fy: a*jo + a/2 - s*y
            BY2 = tp.tile([128, NG], f32)
            nc.vector.tensor_scalar(BY2[:], Y2[:], -SINV, A * 0.5,
                                    op.mult, op.add)
            T1 = tp.tile([128, NG], f32)
            nc.vector.tensor_scalar_mul(T1[:], JO2[:], A)
            nc.vector.tensor_add(BY2[:], BY2[:], T1[:])
            # scatter offsets: k*16384 + jo*128
            KI2 = tp.tile([128, NG], f32)
            nc.gpsimd.iota(KI2[:], pattern=[[128, NG]], base=0,
                           channel_multiplier=1,
                           allow_small_or_imprecise_dtypes=True)
            OFF2 = tp.tile([128, NG], f32)
            nc.vector.tensor_scalar_mul(OFF2[:], KI2[:], float(S * S))
            nc.vector.tensor_scalar_mul(T1[:], JO2[:], float(S))
            nc.vector.tensor_add(OFF2[:], OFF2[:], T1[:])
            OFF2I = tp.tile([128, NG], mybir.dt.int32)
            nc.vector.tensor_copy(OFF2I[:], OFF2[:])
            # offsets for the second r-chunk scatter: +RC*S elements
            OFF2IB = tp.tile([128, NG], mybir.dt.int32)
            nc.vector.tensor_scalar_add(OFF2IB[:], OFF2I[:],
                                        (WIN // RCH) * S)

            IOTA_I = tp.tile([128, S], f32)
            nc.gpsimd.iota(IOTA_I[:], pattern=[[1, S]], base=0,
                           channel_multiplier=0,
                           allow_small_or_imprecise_dtypes=True)
            RIOTA = tp.tile([128, WIN], f32)
            nc.gpsimd.iota(RIOTA[:], pattern=[[1, WIN]], base=0,
                           channel_multiplier=0,
                           allow_small_or_imprecise_dtypes=True)

            # ---- main loop: one group of <=128 heatmaps per iteration ----
            order = ([NG - 1] if NG_REM else []) + list(range(NG_FULL))
            for g in order:
                n = GROUPS[g]
                FX = vp.tile([128, S], f32, tag="fx")      # fx row per hm
                nc.scalar.activation(FX[0:n, :], IOTA_I[0:n, :], derf,
                                     bias=BX2[0:n, g:g + 1], scale=A)
                FY = vp.tile([128, WIN], f32, tag="fy")    # fy row per hm
                nc.scalar.activation(FY[0:n, :], RIOTA[0:n, :], derf,
                                     bias=BY2[0:n, g:g + 1], scale=A)
                nc.vector.tensor_scalar_mul(FY[0:n, :], FY[0:n, :], AMP)

                rc = WIN // RCH
                fyap = FY[0:n, :]
                fxap = FX[0:n, :]
                for r in range(RCH):
                    # one tile per r-chunk: its scatter fires as soon as its
                    # product is done (finer DMA pipelining)
                    GC = mp.tile([128, rc * S], f32, tag="g")
                    in0 = AP(tensor=fyap.tensor,
                             offset=fyap.offset + r * rc,
                             ap=[[fyap.ap[0][0], n], [1, rc], [0, S]])
                    in1 = AP(tensor=fxap.tensor, offset=fxap.offset,
                             ap=[[fxap.ap[0][0], n], [0, rc], [1, S]])
                    nc.vector.tensor_tensor(GC[0:n, :], in0, in1, op.mult)
                    offt = OFF2I if r == 0 else OFF2IB
                    nc.gpsimd.indirect_dma_start(
                        o2d,
                        IndirectOffsetOnAxis(ap=offt[0:n, g:g + 1], axis=0),
                        GC[0:n, :], None)

    nc.compile()
    return nc


def _get_nc():
    if "nc" not in _cache:
        _cache["nc"] = _build()
    return _cache["nc"]


def _run(coords_full, trace=False):
    from concourse.bass_utils import run_bass_kernel_spmd

    coords_full = np.ascontiguousarray(np.asarray(coords_full, dtype=np.float32))
    assert coords_full.shape == (B_TOTAL, 2 * NUM_CLASS)
    nc = _get_nc()
    in_maps = [{"coords": coords_full[i * B_LOC:(i + 1) * B_LOC]}
               for i in range(N_CORES)]
    br = run_bass_kernel_spmd(nc, in_maps, core_ids=list(range(N_CORES)),
                              trace=trace)
    parts = [br.results[i]["out"].reshape(B_LOC, NUM_CLASS, S, S)
             for i in range(N_CORES)]
    full = np.concatenate(parts, axis=0)
    return full, br


def kernel(coords):
    return _run(coords, trace=False)[0]


# revision 7
# speedup vs baseline: 1.4102x; 1.4102x over previous
"""Coord2HeatmapNet Trainium2 kernel.

out[b,c,j,i] = 10*exp(-(((i+.5)/128 - x)^2 + ((j+.5)/128 - y)^2) / (2*(2/128)^2))

Exploited structure:
  * Separable: each heatmap = fy[j] (x) fx[i] outer product.
  * fp32 exp underflows to exactly 0 beyond ~29 px from the peak -> only a
    64-row window per heatmap is nonzero; the pre-zeroed output buffer keeps
    the rest at 0.
  * Derivative_Erf activation = 2/sqrt(pi)*exp(-t^2): one ScalarE op per
    gaussian factor vector.
  * Layout: one heatmap per PARTITION. Partition p of group g holds the whole
    64x128 window of heatmap k=g*128+p as 8192 contiguous floats. The outer
    product is one DVE tensor_tensor with stride-0 broadcasts; the write-out
    is ONE indirect scatter DMA per group (one offset per partition, 32KB
    contiguous per heatmap at its data-dependent window position).
  * coords flat index of x_k is 2k (affine), so per-partition coords load is
    a plain strided DMA.

Sharding: pure data parallel, 8 batches per core across 8 NeuronCores.
"""
import sys

for _p in ("/opt/trn_rl_repo", "/root/.axon_site", "/root/.axon_site/_ro/trn_rl_repo",
           "/root/.axon_site/_ro/pypackages"):
    if _p not in sys.path:
        sys.path.append(_p)

import numpy as np

S = 128
NUM_CLASS = 68
B_TOTAL = 64
N_CORES = 8
B_LOC = B_TOTAL // N_CORES            # 8 batches per core
NHM = B_LOC * NUM_CLASS               # 544 heatmaps per core
WIN = 64                              # window rows per heatmap
NG_FULL = NHM // 128                  # 4 full groups of 128 heatmaps
NG_REM = NHM - NG_FULL * 128          # 32 in the last group
GROUPS = [128] * NG_FULL + ([NG_REM] if NG_REM else [])
FREE = WIN * S                        # 8192 elems (32KB) per heatmap window
SIGMA = 2.0 / S
DENOM = 2.0 * SIGMA * SIGMA           # 1/2048
SINV = float(np.sqrt(1.0 / DENOM))    # 45.254834
A = SINV / S
AMP = float(10.0 * np.pi / 4.0)
OUT_ELEMS = NHM * S * S
RCH = 2                               # DVE product ops per group (r-chunks)

_cache = {}


def _build():
    import concourse.bass as bass
    import concourse.tile as tile
    from concourse import bacc, mybir
    from concourse.bass import IndirectOffsetOnAxis
    from concourse.bass_types import AP

    f32 = mybir.dt.float32
    nc = bacc.Bacc("TRN2", target_bir_lowering=False, debug=False,
                   num_devices=N_CORES)

    coords = nc.dram_tensor("coords", [B_LOC, 2 * NUM_CLASS], f32,
                            kind="ExternalInput")
    out = nc.dram_tensor("out", [OUT_ELEMS], f32, kind="ExternalOutput")
    o2d = out.ap().rearrange("(a b) -> a b", b=1)
    cflat = coords.ap().rearrange("b f -> (b f)")

    derf = mybir.ActivationFunctionType.Derivative_Erf
    op = mybir.AluOpType
    NG = len(GROUPS)

    with tile.TileContext(nc) as tc:
        with tc.tile_pool(name="tabs", bufs=1) as tp, \
             tc.tile_pool(name="main", bufs=4) as mp, \
             tc.tile_pool(name="vecs", bufs=2) as vp:
            # ---- per-heatmap coord tables, partition p = heatmap g*128+p ----
            X2 = tp.tile([128, NG], f32)
            Y2 = tp.tile([128, NG], f32)
            for (t, off) in ((X2, 0), (Y2, 1)):
                # full groups: coords_flat[2*(g*128+p) + off]
                src = AP(tensor=cflat.tensor, offset=off,
                         ap=[[2, 128], [256, NG_FULL]])
                nc.sync.dma_start(t[:, 0:NG_FULL], src)
                if NG_REM:
                    srcr = AP(tensor=cflat.tensor,
                              offset=off + 2 * 128 * NG_FULL,
                              ap=[[2, NG_REM], [256, 1]])
                    nc.sync.dma_start(t[0:NG_REM, NG_FULL:NG], srcr)

            # bias for fx: a/2 - s*x
            BX2 = tp.tile([128, NG], f32)
            nc.vector.tensor_scalar(BX2[:], X2[:], -SINV, A * 0.5,
                                    op.mult, op.add)
            # jo = clamp(rint(128*y) - 32, 0, 64)
            JO2 = tp.tile([128, NG], f32)
            nc.vector.tensor_scalar_mul(JO2[:], Y2[:], float(S))
            JO2I = tp.tile([128, NG], mybir.dt.int32)
            nc.vector.tensor_copy(JO2I[:], JO2[:])
            nc.vector.tensor_copy(JO2[:], JO2I[:])
            nc.vector.tensor_scalar_sub(JO2[:], JO2[:], 32.0)
            nc.vector.tensor_scalar(JO2[:], JO2[:], 0.0, 64.0, op.max, op.min)
            # bias for fy: a*jo + a/2 - s*y
            BY2 = tp.tile([128, NG], f32)
            nc.vector.tensor_scalar(BY2[:], Y2[:], -SINV, A * 0.5,
                                    op.mult, op.add)
            T1 = tp.tile([128, NG], f32)
            nc.vector.tensor_scalar_mul(T1[:], JO2[:], A)
            nc.vector.tensor_add(BY2[:], BY2[:], T1[:])
            # scatter offsets: k*16384 + jo*128
            KI2 = tp.tile([128, NG], f32)
            nc.gpsimd.iota(KI2[:], pattern=[[128, NG]], base=0,
                           channel_multiplier=1,
                           allow_small_or_imprecise_dtypes=True)
            OFF2 = tp.tile([128, NG], f32)
            nc.vector.tensor_scalar_mul(OFF2[:], KI2[:], float(S * S))
            nc.vector.tensor_scalar_mul(T1[:], JO2[:], float(S))
            nc.vector.tensor_add(OFF2[:], OFF2[:], T1[:])
            OFF2I = tp.tile([128, NG], mybir.dt.int32)
            nc.vector.tensor_copy(OFF2I[:], OFF2[:])
            # offsets for the second r-chunk scatter: +RC*S elements
            OFF2IB = tp.tile([128, NG], mybir.dt.int32)
            nc.vector.tensor_scalar_add(OFF2IB[:], OFF2I[:],
                                        (WIN // RCH) * S)

            IOTA_I = tp.tile([128, S], f32)
            nc.gpsimd.iota(IOTA_I[:], pattern=[[1, S]], base=0,
                           channel_multiplier=0,
                           allow_small_or_imprecise_dtypes=True)
            RIOTA = tp.tile([128, WIN], f32)
            nc.gpsimd.iota(RIOTA[:], pattern=[[1, WIN]], base=0,
                           channel_multiplier=0,
                           allow_small_or_imprecise_dtypes=True)

            # ---- main loop: one group of <=128 heatmaps per iteration ----
            order = ([NG - 1] if NG_REM else []) + list(range(NG_FULL))
            for g in order:
                n = GROUPS[g]
                FX = vp.tile([128, S], f32, tag="fx")      # fx row per hm
                nc.scalar.activation(FX[0:n, :], IOTA_I[0:n, :], derf,
                                     bias=BX2[0:n, g:g + 1], scale=A)
                FY = vp.tile([128, WIN], f32, tag="fy")    # fy row per hm
                nc.scalar.activation(FY[0:n, :], RIOTA[0:n, :], derf,
                                     bias=BY2[0:n, g:g + 1], scale=A)
                nc.vector.tensor_scalar_mul(FY[0:n, :], FY[0:n, :], AMP)

                rc = WIN // RCH
                fyap = FY[0:n, :]
                fxap = FX[0:n, :]
                G = mp.tile([128, FREE], f32, tag="g")
                for r in range(RCH):
                    in0 = AP(tensor=fyap.tensor,
                             offset=fyap.offset + r * rc,
                             ap=[[fyap.ap[0][0], n], [1, rc], [0, S]])
                    in1 = AP(tensor=fxap.tensor, offset=fxap.offset,
                             ap=[[fxap.ap[0][0], n], [0, rc], [1, S]])
                    nc.vector.tensor_tensor(
                        G[0:n, r * rc * S:(r + 1) * rc * S], in0, in1,
                        op.mult)
                nc.gpsimd.indirect_dma_start(
                    o2d,
                    IndirectOffsetOnAxis(ap=OFF2I[0:n, g:g + 1], axis=0),
                    G[0:n, :], None)

    nc.compile()
    return nc


def _get_nc():
    if "nc" not in _cache:
        _cache["nc"] = _build()
    return _cache["nc"]


def _run(coords_full, trace=False):
    from concourse.bass_utils import run_bass_kernel_spmd

    coords_full = np.ascontiguousarray(np.asarray(coords_full, dtype=np.float32))
    assert coords_full.shape == (B_TOTAL, 2 * NUM_CLASS)
    nc = _get_nc()
    in_maps = [{"coords": coords_full[i * B_LOC:(i + 1) * B_LOC]}
               for i in range(N_CORES)]
    br = run_bass_kernel_spmd(nc, in_maps, core_ids=list(range(N_CORES)),
                              trace=trace)
    parts = [br.results[i]["out"].reshape(B_LOC, NUM_CLASS, S, S)
             for i in range(N_CORES)]
    full = np.concatenate(parts, axis=0)
    return full, br


def kernel(coords):
    return _run(coords, trace=False)[0]
